# revision 26
# baseline (speedup 1.0000x reference)
"""AttentionGate kernel for Trainium2 (8 NeuronCores, data-parallel over batch).

Per core (batch element b):
  kp = k @ M (M = 8*Wk@Wq^T) ; vp = v @ Wv + bv
  scores^T = kp @ q^T / (8*sqrt(D))  (q-projection folded into M)
  attn = exp(scores + e_k)  (e_k = k@Wk@bq/sqrt(D) bias; bk cancels)
  out = (attn @ vp) / rowsum(attn)

v3:
  - scores matmuls fp8e4 DoubleRow (feature-block pairs): kp fp8 (x8 via
    M), q cast bf16->fp8 on ACT. 128 DR MMs instead of 256 bf16.
  - All HBM loads are HWDGE f32 (ACT + SP rings) with engine casts to
    bf16; no SWDGE casting loads, so loads are NOT serialized against
    the xbar transposes (Tile serializes SWDGE vs transpose).
  - Weights load/cast/transpose in d-halves; build_M dt-outer so PE
    starts ~4us in.
  - rowsum free-major (ones-stationary, [1,512] per chunk, 16 MMs) +
    4 tiny PE transposes, instead of 256 N=1 MMs.
  - scores blocks interleaved between attn qi groups so ACT exp hides
    under PE attn work; per-128-row output stores (short tail).
"""

import numpy as np

B, S, D = 8, 2048, 512
P = 128
HT = D // P            # 4 feature tiles
ST = S // P            # 16 sequence tiles
QC = 512               # q-chunk (moving free dim) for scores
NQC = S // QC          # 4 q chunks
SCALE = 1.0 / np.sqrt(np.float32(D))
KP_SCALE = 8.0         # kp pre-scale so fp8e4 sees ~unit-std values

_NC = None
PHASE_MARKS = []


def build_nc(repeat=1):
    import concourse.bass as bass
    import concourse.mybir as mybir
    import concourse.tile as tile
    from concourse import bacc
    from concourse._compat import get_trn_type
    from concourse.tile_rust import add_dep_helper

    f32 = mybir.dt.float32
    bf16 = mybir.dt.bfloat16
    fp8 = mybir.dt.float8e4
    DR = mybir.MatmulPerfMode.DoubleRow
    Copy = mybir.ActivationFunctionType.Copy
    Exp = mybir.ActivationFunctionType.Exp

    nc = bacc.Bacc(get_trn_type() or "TRN2", target_bir_lowering=False, debug=False)
    q_h = nc.dram_tensor("q", [S, D], f32, kind="ExternalInput")
    k_h = nc.dram_tensor("k", [S, D], f32, kind="ExternalInput")
    v_h = nc.dram_tensor("v", [S, D], f32, kind="ExternalInput")
    Wq_h = nc.dram_tensor("Wq", [D, D], f32, kind="ExternalInput")
    bq_h = nc.dram_tensor("bq", [D], f32, kind="ExternalInput")
    Wk_h = nc.dram_tensor("Wk", [D, D], f32, kind="ExternalInput")
    bk_h = nc.dram_tensor("bk", [D], f32, kind="ExternalInput")
    Wv_h = nc.dram_tensor("Wv", [D, D], f32, kind="ExternalInput")
    bv_h = nc.dram_tensor("bv", [D], f32, kind="ExternalInput")
    out_h = nc.dram_tensor("out", [S, D], f32, kind="ExternalOutput")

    PHASE_MARKS.clear()

    def mark(label):
        nm = nc.get_next_instruction_name()
        PHASE_MARKS.append((int(nm.split("-")[1]), label))

    with tile.TileContext(nc) as tc:
        with (
            tc.tile_pool(name="consts", bufs=1) as consts,
            tc.tile_pool(name="nat", bufs=3) as nat,
            tc.tile_pool(name="tch", bufs=2) as tch,
            tc.tile_pool(name="big", bufs=1) as big,
            tc.tile_pool(name="expp", bufs=3) as expp,
            tc.tile_pool(name="outp", bufs=2) as outp,
            tc.tile_pool(name="small", bufs=2) as small,
            tc.tile_pool(name="ps_mm", bufs=3, space="PSUM") as ps_mm,
            tc.tile_pool(name="ps_out", bufs=2, space="PSUM") as ps_out,
            tc.tile_pool(name="ps_rs", bufs=2, space="PSUM") as ps_rs,
        ):
            for _rep in range(repeat):
                wkT = consts.tile([P, HT, HT, P], bf16, tag="wkT")
                wqT = consts.tile([P, HT, HT, P], bf16, tag="wqT")
                M_f8 = consts.tile([P, HT, D], fp8, tag="Msb")
                u_bf = consts.tile([P, HT], bf16, tag="ubf")
                e_stage = consts.tile([P, ST], f32, tag="est")
                ones_bf = consts.tile([P, 1], bf16, tag="ones")
                nc.vector.memset(ones_bf, 1.0)
                ident1 = consts.tile([1, 1], f32, tag="ident")
                nc.vector.memset(ident1, 1.0)

                warm_in = consts.tile([P, 64], bf16, tag="warm")
                nc.vector.memset(warm_in, 0.0)
                q_f8 = big.tile([P, NQC, HT, HT, P], fp8, tag="qf8")
                kp_f8 = big.tile([P, HT, S], fp8, tag="kpf8")
                vp = big.tile([P, ST, D], bf16, tag="vp")

                # ---- SWDGE casting loads (Pool) + SP xbars, explicitly
                # group-ordered: xbars serialize against ALL other DMAs
                # (~2us handoff per alternation), so loads and transposes
                # are forced into batches via explicit deps.
                def load_W(W_h, i, pool=None, bufs=1):
                    if pool is None:
                        w_sb = consts.tile([P, HT, D], bf16, tag=f"w{i}",
                                           name=f"w_sb{i}")
                    else:
                        w_sb = pool.tile([P, HT, D], bf16, tag="nat1",
                                         bufs=bufs, name=f"w_sb{i}")
                    ld = nc.gpsimd.dma_start(
                        out=w_sb,
                        in_=W_h[:].rearrange("(c p) j -> p c j", p=P))
                    return w_sb, ld

                def span_load(x_h, sc0, n):
                    xb = nat.tile([P, n * HT, D], bf16, tag=f"nat{n}",
                                  bufs=(5 if n == 1 else 2),
                                  name=f"xb_{sc0}_{n}")
                    ld = nc.gpsimd.dma_start(
                        out=xb,
                        in_=x_h[sc0 * QC:(sc0 + n) * QC, :].rearrange(
                            "(c p) j -> p c j", p=P),
                    )
                    return xb, ld

                def span_trans(xb, n, tag, sc0, bufs=None):
                    if bufs is None:
                        bufs = 2 if n == 1 else 1
                    if tag == "qT":
                        bufs = 1
                    xT = tch.tile([P, n, HT, HT, P], bf16, tag=f"{tag}{n}",
                                  bufs=bufs, name=f"{tag}T_{sc0}")
                    tr = nc.sync.dma_start_transpose(out=xT, in_=xb)
                    return [xT[:, i] for i in range(n)], tr

                def dep(ld, after):
                    add_dep_helper(ld.ins, after.ins, reason="dma group order")

                # G1 loads: wk wq k0 q0 k1 (frontload all early-needed)
                wk, wk_ld = load_W(Wk_h, 1, pool=nat, bufs=5)
                wq, wq_ld = load_W(Wq_h, 0, pool=nat, bufs=5)
                k0_b, k0_ld = span_load(k_h, 0, 1)
                q0_b, q0_ld = span_load(q_h, 0, 1)
                k1_b, k1_ld = span_load(k_h, 1, 1)
                bq_sb = consts.tile([P, HT], f32, tag="bq")
                nc.gpsimd.dma_start(
                    out=bq_sb, in_=bq_h[:].rearrange("(di p) -> p di", p=P))
                bv_bcast = consts.tile([P, D], f32, tag="bv")
                nc.gpsimd.dma_start(
                    out=bv_bcast,
                    in_=bass.AP(tensor=bv_h[:].tensor, offset=0,
                                ap=[[0, P], [1, D]]),
                )
                bq_bf = consts.tile([P, HT], bf16, tag="bqb")
                nc.scalar.activation(
                    out=bq_bf, in_=bq_sb, func=Copy, scale=float(SCALE))
                # T1: k0T q0T k1T wkT wqT
                kT0, k0_tr = span_trans(k0_b, 1, "kT", 0)
                qT0, q0_tr = span_trans(q0_b, 1, "qT", 0)
                kT1, k1_tr = span_trans(k1_b, 1, "kT", 1)
                wkT_tr = nc.sync.dma_start_transpose(out=wkT[:, :, :, :],
                                                     in_=wk)
                wqT_tr = nc.sync.dma_start_transpose(out=wqT[:, :, :, :],
                                                     in_=wq)
                T1 = [k0_tr, q0_tr, k1_tr, wkT_tr, wqT_tr]
                # G2 loads: k23 wv v01 (after T1)
                k23_b, k23_ld = span_load(k_h, 2, 2)
                wv_bf, wv_ld = load_W(Wv_h, 2)
                v01_b, v01_ld = span_load(v_h, 0, 2)
                G2 = [k23_ld, wv_ld, v01_ld]
                for ld in G2:
                    for tr in T1:
                        dep(ld, tr)
                # T2: k23T v01T
                kT23, k23_tr = span_trans(k23_b, 2, "kT", 2)
                vT01, v01_tr = span_trans(v01_b, 2, "vT", 0, bufs=2)
                T2 = [k23_tr, v01_tr]
                for tr in T2:
                    for ld in G2:
                        add_dep_helper(tr.ins, ld.ins, reason="dma group")
                # G3 loads: q1 v23
                q1_b, q1_ld = span_load(q_h, 1, 1)
                v23_b, v23_ld = span_load(v_h, 2, 2)
                G3 = [q1_ld, v23_ld]
                for ld in G3:
                    for tr in T2:
                        dep(ld, tr)
                # T3: q1T v23T
                qT1, q1_tr = span_trans(q1_b, 1, "qT", 1)
                vT23, v23_tr = span_trans(v23_b, 2, "vT", 2, bufs=2)
                T3 = [q1_tr, v23_tr]
                for tr in T3:
                    for ld in G3:
                        add_dep_helper(tr.ins, ld.ins, reason="dma group")
                # G4: q23 | T4: q23T
                q23_b, q23_ld = span_load(q_h, 2, 2)
                for tr in T3:
                    dep(q23_ld, tr)
                qT23, _ = span_trans(q23_b, 2, "qT", 2)

                kTs = kT0 + kT1 + kT23
                qTs = qT0 + qT1 + qT23
                vT = vT01 + vT23

                # k chunks cast to fp8 for the DoubleRow kmh projection
                k_f8 = big.tile([P, NQC, HT, HT, P], fp8, tag="kf8")

                def kf8_cast(sc):
                    nc.vector.tensor_copy(out=k_f8[:, sc], in_=kTs[sc])

                # PE warmup: keep the PE HAM streak alive through the
                # startup DMA window so build_M runs at full clock.
                # Each dummy group is released by a successive DMA
                # completion; results are discarded.
                warm_anchors = [wk_ld, wq_ld, k0_ld, q0_ld, wkT_tr, wqT_tr]
                for wi, anch in enumerate(warm_anchors):
                    ps_w = ps_rs.tile([1, 64], f32, tag="ps_u", bufs=1,
                                      name=f"ps_w{wi}")
                    for wj in range(4):
                        mm = nc.tensor.matmul(
                            ps_w, ones_bf, warm_in,
                            start=(wj == 0), stop=(wj == 3))
                        if wj == 0:
                            add_dep_helper(mm.ins, anch.ins,
                                           reason="pe warmup pacing")

                def qf8_cast(qc):
                    # x4 here and x2 on kp = x8 total (exp scale /8)
                    nc.scalar.activation(
                        out=q_f8[:, qc], in_=qTs[qc], func=Copy, scale=4.0)

                # ---- PE building blocks ----
                def build_M():
                    pss = []
                    for _i in range(3):
                        ps_mtile = ps_mm.tile([P, D], f32, tag="ps_mm",
                                              name=f"ps_M{_i}")
                        pss.append(ps_mtile)
                    ps_m3 = ps_out.tile([P, D], f32, tag="ps_out",
                                        name="ps_M3")
                    pss.append(ps_m3)
                    for dt in range(HT):
                        for bt in range(HT):
                            nc.tensor.matmul(
                                pss[bt], wkT[:, bt, dt, :], wqT[:, :, dt, :],
                                start=(dt == 0), stop=(dt == HT - 1))
                    for bt in range(HT):
                        nc.vector.tensor_scalar_mul(M_f8[:, bt, :],
                                                    pss[bt], 64.0)

                def build_u():
                    for bt in range(HT):
                        ps = ps_rs.tile([P, 1], f32, tag="ps_u", bufs=1)
                        for dt in range(HT):
                            nc.tensor.matmul(
                                ps, wkT[:, bt, dt, :], bq_bf[:, dt:dt + 1],
                                start=(dt == 0), stop=(dt == HT - 1))
                        nc.vector.tensor_copy(out=u_bf[:, bt:bt + 1], in_=ps)

                def build_e(sc):
                    kT = k_f8[:, sc]
                    for c in range(HT):
                        st = sc * HT + c
                        ps = ps_rs.tile([P, 1], f32, tag="ps_u", bufs=1)
                        for jb in range(HT):
                            nc.tensor.matmul(
                                ps, kT[:, c, jb, :], u_bf[:, jb:jb + 1],
                                start=(jb == 0), stop=(jb == HT - 1))
                        nc.vector.tensor_copy(
                            out=e_stage[:, st:st + 1], in_=ps)

                def kmh_chunk(sc):
                    kf8_cast(sc)
                    kT8 = k_f8[:, sc]
                    for ht in range(HT):
                        ps = ps_mm.tile([P, QC], f32, tag="ps_mm")
                        for bp in range(HT // 2):
                            nc.tensor.matmul(
                                ps,
                                M_f8[:, 2 * bp:2 * bp + 2,
                                     ht * P:(ht + 1) * P],
                                kT8[:, :, 2 * bp:2 * bp + 2,
                                    :].transpose([0, 2, 1, 3]),
                                start=(bp == 0),
                                stop=(bp == HT // 2 - 1),
                                perf_mode=DR,
                            )
                        # M x64 and q x4: scale kp by 2/64 so total kp*q
                        # scale is x8 (exp applies /8)
                        nc.vector.tensor_scalar_mul(
                            kp_f8[:, ht, sc * QC:(sc + 1) * QC], ps,
                            2.0 / 64.0)

                def vp_chunk(sc):
                    for i in range(HT):
                        st = sc * HT + i
                        ps = ps_mm.tile([P, D], f32, tag="ps_mm")
                        for hi in range(HT):
                            nc.tensor.matmul(
                                ps,
                                vT[sc][:, i, hi, :],
                                wv_bf[:, hi, :],
                                start=(hi == 0),
                                stop=(hi == HT - 1),
                            )
                        nc.vector.tensor_add(vp[:, st, :], ps, bv_bcast)

                def scores_block(qc, kc, ex):
                    for kt in range(kc * HT, (kc + 1) * HT):
                        ps = ps_mm.tile([P, QC], f32, tag="ps_mm")
                        for dp in range(HT // 2):
                            nc.tensor.matmul(
                                ps,
                                kp_f8[:, 2 * dp:2 * dp + 2,
                                      kt * P:(kt + 1) * P],
                                q_f8[:, qc, :, 2 * dp:2 * dp + 2,
                                     :].transpose([0, 2, 1, 3]),
                                start=(dp == 0),
                                stop=(dp == HT // 2 - 1),
                                perf_mode=DR,
                            )
                        nc.scalar.activation(
                            out=ex[:, kt, :],
                            in_=ps,
                            func=Exp,
                            scale=float(SCALE / KP_SCALE),
                            bias=e_stage[:, kt:kt + 1],
                        )

                rss = {}
                rcs = {}

                def rowsum_part(qc, ex, kt0, kt1, start, stop):
                    if start:
                        rss[qc] = ps_rs.tile([1, QC], f32, tag="ps_rs",
                                             bufs=1, name=f"ps_rs{qc}")
                    ps_r = rss[qc]
                    for kt in range(kt0, kt1):
                        nc.tensor.matmul(
                            ps_r, ones_bf, ex[:, kt, :],
                            start=(start and kt == kt0),
                            stop=(stop and kt == kt1 - 1),
                        )
                    if not stop:
                        return
                    rs_sb = small.tile([1, QC], f32, tag="rs", bufs=1)
                    nc.vector.tensor_copy(out=rs_sb, in_=ps_r)
                    ps_t = ps_rs.tile([P, HT], f32, tag="ps_t", bufs=1)
                    for qi in range(HT):
                        nc.tensor.transpose(
                            out=ps_t[:, qi:qi + 1],
                            in_=rs_sb[:, qi * P:(qi + 1) * P],
                            identity=ident1)
                    rc = small.tile([P, HT], f32, tag="rc", bufs=2,
                                    name=f"rc{qc}")
                    nc.vector.reciprocal(rc, ps_t)
                    rcs[qc] = rc

                obs = {}

                def attn_qi_part(qc, qi, ex, kt0, kt1, first, last):
                    if qi == 0 and first:
                        ob_t = outp.tile([P, HT, D], f32, tag="ob",
                                         name=f"ob{qc}")
                        obs[qc] = ob_t
                    ob = obs[qc]
                    ps_o = ps_out.tile([P, D], f32, tag="ps_out")
                    for kt in range(kt0, kt1):
                        nc.tensor.matmul(
                            ps_o, ex[:, kt, qi * P:(qi + 1) * P],
                            vp[:, kt, :],
                            start=(kt == kt0), stop=(kt == kt1 - 1),
                        )
                    if first and not last:
                        nc.vector.tensor_copy(out=ob[:, qi, :], in_=ps_o)
                        return
                    if last and not first:
                        nc.vector.tensor_add(ob[:, qi, :], ps_o, ob[:, qi, :])
                        nc.vector.tensor_scalar_mul(
                            ob[:, qi, :], ob[:, qi, :], rcs[qc][:, qi:qi + 1])
                    else:
                        nc.vector.tensor_scalar_mul(
                            ob[:, qi, :], ps_o, rcs[qc][:, qi:qi + 1])
                    nc.sync.dma_start(
                        out=out_h[(qc * HT + qi) * P:
                                  (qc * HT + qi + 1) * P, :].rearrange(
                            "(c p) j -> p c j", p=P),
                        in_=ob[:, qi:qi + 1, :])

                def attn_full(qc, ex):
                    for qi in range(HT):
                        attn_qi_part(qc, qi, ex, 0, ST, True, True)

                # ---- PE wavefront (arrival-ordered) ----
                mark("build_M")
                build_M()
                mark("kmh0")
                kmh_chunk(0)
                mark("build_u")
                build_u()
                mark("kmh1")
                kmh_chunk(1)
                mark("e0")
                build_e(0)
                qf8_cast(0)
                ex0 = expp.tile([P, ST, QC], bf16, tag="ex")
                mark("sc(0,0)")
                scores_block(0, 0, ex0)
                mark("e1")
                build_e(1)
                mark("sc(0,1)")
                scores_block(0, 1, ex0)
                mark("kmh2")
                kmh_chunk(2)
                mark("e2")
                build_e(2)
                mark("sc(0,2)")
                scores_block(0, 2, ex0)
                mark("kmh3")
                kmh_chunk(3)
                mark("e3")
                build_e(3)
                mark("sc(0,3)")
                scores_block(0, 3, ex0)
                mark("vp01")
                vp_chunk(0)
                vp_chunk(1)
                mark("rs0h1")
                rowsum_part(0, ex0, 0, ST // 2, True, False)
                mark("attn0h1")
                for qi in range(HT):
                    attn_qi_part(0, qi, ex0, 0, ST // 2, True, False)
                qf8_cast(1)
                ex1 = expp.tile([P, ST, QC], bf16, tag="ex")
                mark("vp23")
                vp_chunk(2)
                vp_chunk(3)
                mark("sc(1,01)")
                for kc in (0, 1):
                    scores_block(1, kc, ex1)
                mark("rs0h2")
                rowsum_part(0, ex0, ST // 2, ST, False, True)
                mark("sc(1,23)")
                for kc in (2, 3):
                    scores_block(1, kc, ex1)
                mark("attn0h2")
                for qi in range(HT):
                    attn_qi_part(0, qi, ex0, ST // 2, ST, False, True)
                mark("rowsum1")
                rowsum_part(1, ex1, 0, ST, True, True)
                qf8_cast(2)
                ex2 = expp.tile([P, ST, QC], bf16, tag="ex")
                mark("attn1+sc2")
                for qi in range(HT):
                    attn_qi_part(1, qi, ex1, 0, ST, True, True)
                    scores_block(2, qi, ex2)
                    if qi == 1:
                        rowsum_part(2, ex2, 0, ST // 2, True, False)
                mark("rs2h2")
                rowsum_part(2, ex2, ST // 2, ST, False, True)
                qf8_cast(3)
                ex3 = expp.tile([P, ST, QC], bf16, tag="ex")
                mark("attn2+sc3")
                for qi in range(HT):
                    attn_qi_part(2, qi, ex2, 0, ST, True, True)
                    scores_block(3, qi, ex3)
                    if qi == 1:
                        rowsum_part(3, ex3, 0, ST // 2, True, False)
                mark("rs3h2")
                rowsum_part(3, ex3, ST // 2, ST, False, True)
                mark("attn3")
                attn_full(3, ex3)

    mark('end')
    nc.compile()
    return nc


def _get_nc():
    global _NC
    if _NC is None:
        _NC = build_nc()
    return _NC


def build_in_maps(q, k, v, Wq, bq, Wk, bk, Wv, bv):
    in_maps = []
    for b in range(B):
        in_maps.append({
            "q": np.ascontiguousarray(q[b], dtype=np.float32),
            "k": np.ascontiguousarray(k[b], dtype=np.float32),
            "v": np.ascontiguousarray(v[b], dtype=np.float32),
            "Wq": np.ascontiguousarray(Wq, dtype=np.float32),
            "bq": np.ascontiguousarray(bq, dtype=np.float32),
            "Wk": np.ascontiguousarray(Wk, dtype=np.float32),
            "bk": np.ascontiguousarray(bk, dtype=np.float32),
            "Wv": np.ascontiguousarray(Wv, dtype=np.float32),
            "bv": np.ascontiguousarray(bv, dtype=np.float32),
        })
    return in_maps


def kernel(q, k, v, Wq, bq, Wk, bk, Wv, bv):
    from concourse.bass_utils import run_bass_kernel_spmd

    nc = _get_nc()
    in_maps = build_in_maps(q, k, v, Wq, bq, Wk, bk, Wv, bv)
    res = run_bass_kernel_spmd(nc, in_maps, core_ids=list(range(B)))
    return np.stack([r["out"] for r in res.results], axis=0)


# revision 37
# speedup vs baseline: 1.0075x; 1.0075x over previous
"""AttentionGate kernel for Trainium2 (8 NeuronCores, data-parallel over batch).

Per core (batch element b):
  kp = k @ M (M = 8*Wk@Wq^T) ; vp = v @ Wv + bv
  scores^T = kp @ q^T / (8*sqrt(D))  (q-projection folded into M)
  attn = exp(scores + e_k)  (e_k = k@Wk@bq/sqrt(D) bias; bk cancels)
  out = (attn @ vp) / rowsum(attn)

v3:
  - scores matmuls fp8e4 DoubleRow (feature-block pairs): kp fp8 (x8 via
    M), q cast bf16->fp8 on ACT. 128 DR MMs instead of 256 bf16.
  - All HBM loads are HWDGE f32 (ACT + SP rings) with engine casts to
    bf16; no SWDGE casting loads, so loads are NOT serialized against
    the xbar transposes (Tile serializes SWDGE vs transpose).
  - Weights load/cast/transpose in d-halves; build_M dt-outer so PE
    starts ~4us in.
  - rowsum free-major (ones-stationary, [1,512] per chunk, 16 MMs) +
    4 tiny PE transposes, instead of 256 N=1 MMs.
  - scores blocks interleaved between attn qi groups so ACT exp hides
    under PE attn work; per-128-row output stores (short tail).
"""

import numpy as np

B, S, D = 8, 2048, 512
P = 128
HT = D // P            # 4 feature tiles
ST = S // P            # 16 sequence tiles
QC = 512               # q-chunk (moving free dim) for scores
NQC = S // QC          # 4 q chunks
SCALE = 1.0 / np.sqrt(np.float32(D))
KP_SCALE = 8.0         # kp pre-scale so fp8e4 sees ~unit-std values

_NC = None
PHASE_MARKS = []


def build_nc(repeat=1):
    import concourse.bass as bass
    import concourse.mybir as mybir
    import concourse.tile as tile
    from concourse import bacc
    from concourse._compat import get_trn_type
    from concourse.tile_rust import add_dep_helper

    f32 = mybir.dt.float32
    bf16 = mybir.dt.bfloat16
    fp8 = mybir.dt.float8e4
    DR = mybir.MatmulPerfMode.DoubleRow
    Copy = mybir.ActivationFunctionType.Copy
    Exp = mybir.ActivationFunctionType.Exp

    nc = bacc.Bacc(get_trn_type() or "TRN2", target_bir_lowering=False, debug=False)
    q_h = nc.dram_tensor("q", [S, D], f32, kind="ExternalInput")
    k_h = nc.dram_tensor("k", [S, D], f32, kind="ExternalInput")
    v_h = nc.dram_tensor("v", [S, D], f32, kind="ExternalInput")
    Wq_h = nc.dram_tensor("Wq", [D, D], f32, kind="ExternalInput")
    bq_h = nc.dram_tensor("bq", [D], f32, kind="ExternalInput")
    Wk_h = nc.dram_tensor("Wk", [D, D], f32, kind="ExternalInput")
    bk_h = nc.dram_tensor("bk", [D], f32, kind="ExternalInput")
    Wv_h = nc.dram_tensor("Wv", [D, D], f32, kind="ExternalInput")
    bv_h = nc.dram_tensor("bv", [D], f32, kind="ExternalInput")
    out_h = nc.dram_tensor("out", [S, D], f32, kind="ExternalOutput")

    PHASE_MARKS.clear()

    def mark(label):
        nm = nc.get_next_instruction_name()
        PHASE_MARKS.append((int(nm.split("-")[1]), label))

    with tile.TileContext(nc) as tc:
        with (
            tc.tile_pool(name="consts", bufs=1) as consts,
            tc.tile_pool(name="nat", bufs=3) as nat,
            tc.tile_pool(name="tch", bufs=2) as tch,
            tc.tile_pool(name="big", bufs=1) as big,
            tc.tile_pool(name="expp", bufs=3) as expp,
            tc.tile_pool(name="outp", bufs=2) as outp,
            tc.tile_pool(name="small", bufs=2) as small,
            tc.tile_pool(name="ps_mm", bufs=3, space="PSUM") as ps_mm,
            tc.tile_pool(name="ps_out", bufs=2, space="PSUM") as ps_out,
            tc.tile_pool(name="ps_rs", bufs=2, space="PSUM") as ps_rs,
        ):
            for _rep in range(repeat):
                wkT = consts.tile([P, HT, HT, P], bf16, tag="wkT")
                wqT = consts.tile([P, HT, HT, P], bf16, tag="wqT")
                M_f8 = consts.tile([P, HT, D], fp8, tag="Msb")
                u_bf = consts.tile([P, HT], bf16, tag="ubf")
                e_stage = consts.tile([P, ST], f32, tag="est")
                ones_bf = consts.tile([P, 1], bf16, tag="ones")
                nc.vector.memset(ones_bf, 1.0)
                ident1 = consts.tile([1, 1], f32, tag="ident")
                nc.vector.memset(ident1, 1.0)
                ones128 = consts.tile([P, P], bf16, tag="ones128")
                nc.vector.memset(ones128, 1.0)
                idm = consts.tile([P, P], bf16, tag="idm")
                nc.gpsimd.affine_select(
                    out=idm, in_=ones128, pattern=[[-1, P]],
                    compare_op=mybir.AluOpType.is_equal, fill=0.0,
                    base=0, channel_multiplier=1)

                warm_in = consts.tile([P, 64], bf16, tag="warm")
                nc.vector.memset(warm_in, 0.0)
                q_f8 = big.tile([P, NQC, HT, HT, P], fp8, tag="qf8")
                kp_f8 = big.tile([P, HT, S], fp8, tag="kpf8")
                vp = big.tile([P, ST, D], bf16, tag="vp")

                # ---- SWDGE casting loads (Pool) + SP xbars, explicitly
                # group-ordered: xbars serialize against ALL other DMAs
                # (~2us handoff per alternation), so loads and transposes
                # are forced into batches via explicit deps.
                def load_W(W_h, i, pool=None, bufs=1):
                    if pool is None:
                        w_sb = consts.tile([P, HT, D], bf16, tag=f"w{i}",
                                           name=f"w_sb{i}")
                    else:
                        w_sb = pool.tile([P, HT, D], bf16, tag="nat1",
                                         bufs=bufs, name=f"w_sb{i}")
                    ld = nc.gpsimd.dma_start(
                        out=w_sb,
                        in_=W_h[:].rearrange("(c p) j -> p c j", p=P))
                    return w_sb, ld

                def span_load(x_h, sc0, n):
                    xb = nat.tile([P, n * HT, D], bf16, tag=f"nat{n}",
                                  bufs=(5 if n == 1 else 2),
                                  name=f"xb_{sc0}_{n}")
                    ld = nc.gpsimd.dma_start(
                        out=xb,
                        in_=x_h[sc0 * QC:(sc0 + n) * QC, :].rearrange(
                            "(c p) j -> p c j", p=P),
                    )
                    return xb, ld

                def span_trans(xb, n, tag, sc0, bufs=None):
                    if bufs is None:
                        bufs = 2 if n == 1 else 1
                    if tag == "qT":
                        bufs = 1
                    xT = tch.tile([P, n, HT, HT, P], bf16, tag=f"{tag}{n}",
                                  bufs=bufs, name=f"{tag}T_{sc0}")
                    tr = nc.sync.dma_start_transpose(out=xT, in_=xb)
                    return [xT[:, i] for i in range(n)], tr

                def dep(ld, after):
                    add_dep_helper(ld.ins, after.ins, reason="dma group order")

                # G1 loads: wk wq k0 q0 k1 (frontload all early-needed)
                wk, wk_ld = load_W(Wk_h, 1, pool=nat, bufs=5)
                wq, wq_ld = load_W(Wq_h, 0, pool=nat, bufs=5)
                k0_b, k0_ld = span_load(k_h, 0, 1)
                q0_b, q0_ld = span_load(q_h, 0, 1)
                k1_b, k1_ld = span_load(k_h, 1, 1)
                bq_sb = consts.tile([P, HT], f32, tag="bq")
                nc.gpsimd.dma_start(
                    out=bq_sb, in_=bq_h[:].rearrange("(di p) -> p di", p=P))
                bv_bcast = consts.tile([P, D], f32, tag="bv")
                nc.gpsimd.dma_start(
                    out=bv_bcast,
                    in_=bass.AP(tensor=bv_h[:].tensor, offset=0,
                                ap=[[0, P], [1, D]]),
                )
                bq_bf = consts.tile([P, HT], bf16, tag="bqb")
                nc.scalar.activation(
                    out=bq_bf, in_=bq_sb, func=Copy, scale=float(SCALE))
                # T1: k0T q0T k1T (weights are PE-transposed instead)
                kT0, k0_tr = span_trans(k0_b, 1, "kT", 0)
                qT0, q0_tr = span_trans(q0_b, 1, "qT", 0)
                kT1, k1_tr = span_trans(k1_b, 1, "kT", 1)
                T1 = [k0_tr, q0_tr, k1_tr]
                # G2 loads: k23 wv v01 v23 q1 q23 (after T1);
                # k23/v01/v23 are PE-transposed (no xbar), q1/q23 xbar.
                k23_b, k23_ld = span_load(k_h, 2, 2)
                wv_bf, wv_ld = load_W(Wv_h, 2)
                v01_b, v01_ld = span_load(v_h, 0, 2)
                v23_b, v23_ld = span_load(v_h, 2, 2)
                q1_b, q1_ld = span_load(q_h, 1, 1)
                q23_b, q23_ld = span_load(q_h, 2, 2)
                G2 = [k23_ld, wv_ld, v01_ld, v23_ld, q1_ld, q23_ld]
                for ld in G2:
                    for tr in T1:
                        dep(ld, tr)
                # T2: q1T q23T
                qT1, q1_tr = span_trans(q1_b, 1, "qT", 1)
                qT23, q23_tr = span_trans(q23_b, 2, "qT", 2)
                for tr in (q1_tr, q23_tr):
                    for ld in G2:
                        add_dep_helper(tr.ins, ld.ins, reason="dma group")

                kTs = kT0 + kT1
                qTs = qT0 + qT1 + qT23
                # vT tiles are written by PE transposes (below)
                vT01_t = tch.tile([P, 2, HT, HT, P], bf16, tag="vT2",
                                  bufs=2, name="vT01_t")
                vT23_t = tch.tile([P, 2, HT, HT, P], bf16, tag="vT2",
                                  bufs=2, name="vT23_t")
                vT = [vT01_t[:, 0], vT01_t[:, 1], vT23_t[:, 0],
                      vT23_t[:, 1]]

                def pe_transpose_k(sc, xb_view):
                    # k chunk natural [p,c,j] -> k_f8[:, sc] via PE + DVE
                    for c in range(HT):
                        o = 4 * (c % 2)
                        pst = ps_rs.tile([P, 8, P], bf16, tag="ps_tr",
                                         bufs=1, name=f"pstk{sc}{c}")
                        for jb in range(HT):
                            nc.tensor.transpose(
                                out=pst[:, o + jb, :],
                                in_=xb_view[:, c, jb * P:(jb + 1) * P],
                                identity=idm)
                        nc.vector.tensor_copy(
                            out=k_f8[:, sc, c, :, :],
                            in_=pst[:, o:o + 4, :])

                def pe_transpose_v(vt_dst, xb_view, nm):
                    # v span natural -> vT tile (bf16) via PE + DVE
                    for i in range(2 * HT):
                        o = 4 * (i % 2)
                        pst = ps_rs.tile([P, 8, P], bf16, tag="ps_tr",
                                         bufs=1, name=f"pstv{nm}{i}")
                        for jb in range(HT):
                            nc.tensor.transpose(
                                out=pst[:, o + jb, :],
                                in_=xb_view[:, i, jb * P:(jb + 1) * P],
                                identity=idm)
                        nc.vector.tensor_copy(
                            out=vt_dst[:, i // HT, i % HT, :, :],
                            in_=pst[:, o:o + 4, :])

                # k chunks cast to fp8 for the DoubleRow kmh projection
                k_f8 = big.tile([P, NQC, HT, HT, P], fp8, tag="kf8")

                def kf8_cast(sc):
                    nc.vector.tensor_copy(out=k_f8[:, sc], in_=kTs[sc])

                # PE warmup: keep the PE HAM streak alive through the
                # startup DMA window so build_M runs at full clock.
                # Each dummy group is released by a successive DMA
                # completion; results are discarded.
                warm_anchors = [wk_ld, wq_ld]
                for wi, anch in enumerate(warm_anchors):
                    ps_w = ps_rs.tile([1, 64], f32, tag="ps_u", bufs=1,
                                      name=f"ps_w{wi}")
                    for wj in range(4):
                        mm = nc.tensor.matmul(
                            ps_w, ones_bf, warm_in,
                            start=(wj == 0), stop=(wj == 3))
                        if wj == 0:
                            add_dep_helper(mm.ins, anch.ins,
                                           reason="pe warmup pacing")

                def qf8_cast(qc):
                    # x4 here and x2 on kp = x8 total (exp scale /8)
                    nc.scalar.activation(
                        out=q_f8[:, qc], in_=qTs[qc], func=Copy, scale=4.0)

                # ---- PE building blocks ----
                def build_M():
                    # PE-transpose wk/wq into wkT/wqT (dt-pipelined with
                    # the M matmuls; PE is otherwise idle at startup)
                    pss = []
                    for _i in range(3):
                        ps_mtile = ps_mm.tile([P, D], f32, tag="ps_mm",
                                              name=f"ps_M{_i}")
                        pss.append(ps_mtile)
                    ps_m3 = ps_out.tile([P, D], f32, tag="ps_out",
                                        name="ps_M3")
                    pss.append(ps_m3)
                    ps_tr = ps_rs.tile([P, 8, P], bf16, tag="ps_tr",
                                       bufs=1, name="ps_tr")
                    ps_tr2 = ps_rs.tile([P, 8, P], bf16, tag="ps_rs",
                                        bufs=1, name="ps_tr2")
                    for dt in range(HT):
                        o = 4 * (dt % 2)
                        for c in range(HT):
                            nc.tensor.transpose(
                                out=ps_tr[:, o + c, :],
                                in_=wk[:, c, dt * P:(dt + 1) * P],
                                identity=idm)
                        for c in range(HT):
                            nc.tensor.transpose(
                                out=ps_tr2[:, o + c, :],
                                in_=wq[:, c, dt * P:(dt + 1) * P],
                                identity=idm)
                        nc.vector.tensor_copy(out=wkT[:, :, dt, :],
                                              in_=ps_tr[:, o:o + 4, :])
                        nc.vector.tensor_copy(out=wqT[:, :, dt, :],
                                              in_=ps_tr2[:, o:o + 4, :])
                        for bt in range(HT):
                            nc.tensor.matmul(
                                pss[bt], wkT[:, bt, dt, :], wqT[:, :, dt, :],
                                start=(dt == 0), stop=(dt == HT - 1))
                    for bt in range(HT):
                        nc.vector.tensor_scalar_mul(M_f8[:, bt, :],
                                                    pss[bt], 64.0)

                def build_u():
                    for bt in range(HT):
                        ps = ps_rs.tile([P, 1], f32, tag="ps_u", bufs=1)
                        for dt in range(HT):
                            nc.tensor.matmul(
                                ps, wkT[:, bt, dt, :], bq_bf[:, dt:dt + 1],
                                start=(dt == 0), stop=(dt == HT - 1))
                        nc.vector.tensor_copy(out=u_bf[:, bt:bt + 1], in_=ps)

                def build_e(sc):
                    kT = k_f8[:, sc]
                    for c in range(HT):
                        st = sc * HT + c
                        ps = ps_rs.tile([P, 1], f32, tag="ps_u", bufs=1)
                        for jb in range(HT):
                            nc.tensor.matmul(
                                ps, kT[:, c, jb, :], u_bf[:, jb:jb + 1],
                                start=(jb == 0), stop=(jb == HT - 1))
                        nc.vector.tensor_copy(
                            out=e_stage[:, st:st + 1], in_=ps)

                def kmh_chunk(sc):
                    if sc < 2:
                        kf8_cast(sc)
                    kT8 = k_f8[:, sc]
                    for ht in range(HT):
                        ps = ps_mm.tile([P, QC], f32, tag="ps_mm")
                        for bp in range(HT // 2):
                            nc.tensor.matmul(
                                ps,
                                M_f8[:, 2 * bp:2 * bp + 2,
                                     ht * P:(ht + 1) * P],
                                kT8[:, :, 2 * bp:2 * bp + 2,
                                    :].transpose([0, 2, 1, 3]),
                                start=(bp == 0),
                                stop=(bp == HT // 2 - 1),
                                perf_mode=DR,
                            )
                        # M x64 and q x4: scale kp by 2/64 so total kp*q
                        # scale is x8 (exp applies /8)
                        nc.vector.tensor_scalar_mul(
                            kp_f8[:, ht, sc * QC:(sc + 1) * QC], ps,
                            2.0 / 64.0)

                def vp_chunk(sc):
                    for i in range(HT):
                        st = sc * HT + i
                        ps = ps_mm.tile([P, D], f32, tag="ps_mm")
                        for hi in range(HT):
                            nc.tensor.matmul(
                                ps,
                                vT[sc][:, i, hi, :],
                                wv_bf[:, hi, :],
                                start=(hi == 0),
                                stop=(hi == HT - 1),
                            )
                        nc.vector.tensor_add(vp[:, st, :], ps, bv_bcast)

                def scores_block(qc, kc, ex):
                    for kt in range(kc * HT, (kc + 1) * HT):
                        ps = ps_mm.tile([P, QC], f32, tag="ps_mm")
                        for dp in range(HT // 2):
                            nc.tensor.matmul(
                                ps,
                                kp_f8[:, 2 * dp:2 * dp + 2,
                                      kt * P:(kt + 1) * P],
                                q_f8[:, qc, :, 2 * dp:2 * dp + 2,
                                     :].transpose([0, 2, 1, 3]),
                                start=(dp == 0),
                                stop=(dp == HT // 2 - 1),
                                perf_mode=DR,
                            )
                        nc.scalar.activation(
                            out=ex[:, kt, :],
                            in_=ps,
                            func=Exp,
                            scale=float(SCALE / KP_SCALE),
                            bias=e_stage[:, kt:kt + 1],
                        )

                rss = {}
                rcs = {}

                def rowsum_part(qc, ex, kt0, kt1, start, stop):
                    if start:
                        rss[qc] = ps_rs.tile([1, QC], f32, tag="ps_rs",
                                             bufs=1, name=f"ps_rs{qc}")
                    ps_r = rss[qc]
                    for kt in range(kt0, kt1):
                        nc.tensor.matmul(
                            ps_r, ones_bf, ex[:, kt, :],
                            start=(start and kt == kt0),
                            stop=(stop and kt == kt1 - 1),
                        )
                    if not stop:
                        return
                    rs_sb = small.tile([1, QC], f32, tag="rs", bufs=1)
                    nc.vector.tensor_copy(out=rs_sb, in_=ps_r)
                    ps_t = ps_rs.tile([P, HT], f32, tag="ps_u", bufs=1)
                    for qi in range(HT):
                        nc.tensor.transpose(
                            out=ps_t[:, qi:qi + 1],
                            in_=rs_sb[:, qi * P:(qi + 1) * P],
                            identity=ident1)
                    rc = small.tile([P, HT], f32, tag="rc", bufs=2,
                                    name=f"rc{qc}")
                    nc.vector.reciprocal(rc, ps_t)
                    rcs[qc] = rc

                obs = {}

                def attn_qi_part(qc, qi, ex, kt0, kt1, first, last):
                    if qi == 0 and first:
                        ob_t = outp.tile([P, HT, D], f32, tag="ob",
                                         name=f"ob{qc}")
                        obs[qc] = ob_t
                    ob = obs[qc]
                    ps_o = ps_out.tile([P, D], f32, tag="ps_out")
                    for kt in range(kt0, kt1):
                        nc.tensor.matmul(
                            ps_o, ex[:, kt, qi * P:(qi + 1) * P],
                            vp[:, kt, :],
                            start=(kt == kt0), stop=(kt == kt1 - 1),
                        )
                    if first and not last:
                        nc.vector.tensor_copy(out=ob[:, qi, :], in_=ps_o)
                        return
                    if last and not first:
                        nc.vector.tensor_add(ob[:, qi, :], ps_o, ob[:, qi, :])
                        nc.vector.tensor_scalar_mul(
                            ob[:, qi, :], ob[:, qi, :], rcs[qc][:, qi:qi + 1])
                    else:
                        nc.vector.tensor_scalar_mul(
                            ob[:, qi, :], ps_o, rcs[qc][:, qi:qi + 1])
                    nc.sync.dma_start(
                        out=out_h[(qc * HT + qi) * P:
                                  (qc * HT + qi + 1) * P, :].rearrange(
                            "(c p) j -> p c j", p=P),
                        in_=ob[:, qi:qi + 1, :])

                def attn_full(qc, ex):
                    for qi in range(HT):
                        attn_qi_part(qc, qi, ex, 0, ST, True, True)

                # ---- PE wavefront (arrival-ordered) ----
                mark("build_M")
                build_M()
                mark("kmh0")
                kmh_chunk(0)
                mark("build_u")
                build_u()
                mark("kmh1")
                kmh_chunk(1)
                mark("e0")
                build_e(0)
                qf8_cast(0)
                ex0 = expp.tile([P, ST, QC], bf16, tag="ex")
                mark("sc(0,0)")
                scores_block(0, 0, ex0)
                mark("e1")
                build_e(1)
                mark("sc(0,1)")
                scores_block(0, 1, ex0)
                mark("peT_k23")
                pe_transpose_k(2, k23_b[:, 0:HT, :])
                pe_transpose_k(3, k23_b[:, HT:2 * HT, :])
                mark("kmh2")
                kmh_chunk(2)
                mark("e2")
                build_e(2)
                mark("sc(0,2)")
                scores_block(0, 2, ex0)
                mark("kmh3")
                kmh_chunk(3)
                mark("e3")
                build_e(3)
                mark("sc(0,3)")
                scores_block(0, 3, ex0)
                mark("peT_v01")
                pe_transpose_v(vT01_t, v01_b, "a")
                mark("vp01")
                vp_chunk(0)
                vp_chunk(1)
                mark("peT_v23")
                pe_transpose_v(vT23_t, v23_b, "b")
                mark("rs0h1")
                rowsum_part(0, ex0, 0, ST // 2, True, False)
                mark("attn0h1")
                for qi in range(HT):
                    attn_qi_part(0, qi, ex0, 0, ST // 2, True, False)
                qf8_cast(1)
                ex1 = expp.tile([P, ST, QC], bf16, tag="ex")
                mark("vp23")
                vp_chunk(2)
                vp_chunk(3)
                mark("sc(1,01)")
                for kc in (0, 1):
                    scores_block(1, kc, ex1)
                mark("rs0h2")
                rowsum_part(0, ex0, ST // 2, ST, False, True)
                mark("sc(1,23)")
                for kc in (2, 3):
                    scores_block(1, kc, ex1)
                mark("attn0h2")
                for qi in range(HT):
                    attn_qi_part(0, qi, ex0, ST // 2, ST, False, True)
                mark("rowsum1")
                rowsum_part(1, ex1, 0, ST, True, True)
                qf8_cast(2)
                ex2 = expp.tile([P, ST, QC], bf16, tag="ex")
                mark("attn1+sc2")
                for qi in range(HT):
                    attn_qi_part(1, qi, ex1, 0, ST, True, True)
                    scores_block(2, qi, ex2)
                    if qi == 1:
                        rowsum_part(2, ex2, 0, ST // 2, True, False)
                mark("rs2h2")
                rowsum_part(2, ex2, ST // 2, ST, False, True)
                qf8_cast(3)
                ex3 = expp.tile([P, ST, QC], bf16, tag="ex")
                mark("attn2+sc3")
                for qi in range(HT):
                    attn_qi_part(2, qi, ex2, 0, ST, True, True)
                    scores_block(3, qi, ex3)
                    if qi == 1:
                        rowsum_part(3, ex3, 0, ST // 2, True, False)
                mark("rs3h2")
                rowsum_part(3, ex3, ST // 2, ST, False, True)
                mark("attn3")
                attn_full(3, ex3)

    mark('end')
    nc.compile()
    return nc


def _get_nc():
    global _NC
    if _NC is None:
        _NC = build_nc()
    return _NC


def build_in_maps(q, k, v, Wq, bq, Wk, bk, Wv, bv):
    in_maps = []
    for b in range(B):
        in_maps.append({
            "q": np.ascontiguousarray(q[b], dtype=np.float32),
            "k": np.ascontiguousarray(k[b], dtype=np.float32),
            "v": np.ascontiguousarray(v[b], dtype=np.float32),
            "Wq": np.ascontiguousarray(Wq, dtype=np.float32),
            "bq": np.ascontiguousarray(bq, dtype=np.float32),
            "Wk": np.ascontiguousarray(Wk, dtype=np.float32),
            "bk": np.ascontiguousarray(bk, dtype=np.float32),
            "Wv": np.ascontiguousarray(Wv, dtype=np.float32),
            "bv": np.ascontiguousarray(bv, dtype=np.float32),
        })
    return in_maps


def kernel(q, k, v, Wq, bq, Wk, bk, Wv, bv):
    from concourse.bass_utils import run_bass_kernel_spmd

    nc = _get_nc()
    in_maps = build_in_maps(q, k, v, Wq, bq, Wk, bk, Wv, bv)
    res = run_bass_kernel_spmd(nc, in_maps, core_ids=list(range(B)))
    return np.stack([r["out"] for r in res.results], axis=0)
